# revision 45
# baseline (speedup 1.0000x reference)
"""Trainium2 Bass kernel for the BF16Indexer sparse-attention problem.

Computes, for B=1, M=2048, H=32, D=128, N=4096:
    logits = einsum('bmhd,bnd->bmhn', q, k)          (fp32 accum)
    o      = einsum('bmhn,bmh->bmn', relu(logits), w) / sqrt(D)

Sharding: M (query tokens) split across 8 cores; k replicated.

Per-core algorithm (M_loc = 256 rows, mh = M_loc*H = 8192):
  - qT  [128=d, mh]     (host-transposed shard of q)
  - kT  [128=d, N]      (host-transposed k, replicated)
  - wblk[128, n_tiles*128]  block-diagonal per-tile weight matrices
  - mm1 (PE):  for each mh-tile t (128 rows = 4 m's x 32 h):
        p1 = qT[:, t].T @ kT[:, chunk]         -> logits [128, 512] fp32 PSUM
  - drain (ACT on even tiles / DVE on odd): y = relu(scale*p1) -> bf16 SBUF
  - mm2 (PE):  p2[:, chunk] += wblk[:, t].T @ y  accumulated over the 32
        tiles of a group (block-diagonal lhsT routes each tile's 4 m's to
        the right 4 of 128 output partitions)
  - p2 [128=m, n_chunk] fp32 -> SBUF -> DMA to o[m, n]

The whole kernel is one flat software pipeline over (group, n-half, tile)
with mm2 trailing mm1 by DELAY tiles, so the PE streams matmuls
back-to-back (~215ns each) across pass boundaries. Steady state is
PE-bound at ~128 elem/cycle ingest for both matmuls (~220us/core); the
PSUM->SBUF relu drains run concurrently on ACT+DVE (~69% busy each).
PE warm-up matmuls trip the HAM clock gate to 2.4GHz during the initial
DMA loads.

kernel(**inputs) takes the FULL inputs and returns the FULL (1, 2048, 4096)
fp32 output; sharding/gather is host-side marshalling only (no host FLOPs).
Measured: ~242us HW exec per core (8 cores SPMD, PE 92% busy),
rel err 1.8e-3.
"""

import math
import numpy as np
import ml_dtypes

import concourse.bass as bass
import concourse.mybir as mybir
import concourse.tile as tile
from concourse import bacc
from concourse.bass_utils import run_bass_kernel_spmd

# Problem constants (hardcoded per harness contract)
B, M, H, D, N = 1, 2048, 32, 128, 4096
N_CORES = 8
M_LOC = M // N_CORES              # 256 query rows per core
MH = M_LOC * H                    # 8192
N_TILES = MH // 128               # 64 mh-tiles (4 m's each)
SOFTMAX_SCALE = 1.0 / math.sqrt(float(D))


def build_nc(m_loc=M_LOC, n=N, group_tiles=32, n_chunk=1024):
    """Build + compile the per-core bass program.

    group_tiles: mh-tiles per mm2 accumulation group (psum2 has
                 4*group_tiles output partitions).
    n_chunk:     n-columns processed per (group, half) pass; psum2 is
                 [128, n_chunk] fp32 = n_chunk/512 PSUM banks.
    """
    mh = m_loc * H
    n_tiles = mh // 128
    assert n_tiles % group_tiles == 0
    n_groups = n_tiles // group_tiles
    assert n % n_chunk == 0
    n_halves = n // n_chunk
    assert n_chunk % 512 == 0
    c_per_half = n_chunk // 512
    gp = 4 * group_tiles  # output partitions per group

    nc = bacc.Bacc("TRN2", target_bir_lowering=False, debug=False)

    bf16 = mybir.dt.bfloat16
    f32 = mybir.dt.float32

    qT_d = nc.dram_tensor("qT", [128, mh], bf16, kind="ExternalInput")
    kT_d = nc.dram_tensor("kT", [128, n], bf16, kind="ExternalInput")
    wblk_d = nc.dram_tensor("wblk", [128, n_tiles * gp], bf16, kind="ExternalInput")
    o_d = nc.dram_tensor("o", [m_loc, n], f32, kind="ExternalOutput")

    with tile.TileContext(nc) as tc:
        with (
            tc.tile_pool(name="const", bufs=1) as const_pool,
            tc.tile_pool(name="ypool", bufs=5) as ypool,
            tc.tile_pool(name="psum1", bufs=6, space="PSUM") as psum1,
            tc.tile_pool(name="psum2", bufs=2, space="PSUM") as psum2,
            tc.tile_pool(name="ostage", bufs=4) as ostage,
        ):
            qT = const_pool.tile([128, mh], bf16)
            kT = const_pool.tile([128, n], bf16)
            wblk = const_pool.tile([128, n_tiles * gp], bf16)

            wb_n = n_tiles * gp
            nc.sync.dma_start(kT[:, :512], kT_d[:, :512])
            nc.scalar.dma_start(kT[:, 512:1024], kT_d[:, 512:1024])
            # warm the ACT spline tables while DMAs run
            warm = const_pool.tile([128, 1], bf16)
            nc.gpsimd.memset(warm[:], 0)
            nc.scalar.activation(warm[:], warm[:],
                                 mybir.ActivationFunctionType.Relu)

            # warm the PE (HAM un-throttles after ~3.4us of activity) with
            # small matmuls on a zeroed scratch tile while DMAs run
            if n_tiles >= 16:
                wsrc = const_pool.tile([128, 128], bf16)
                nc.gpsimd.memset(wsrc[:], 0)
                wps = psum1.tile([128, 128], f32, tag="p1", name="warm_ps")
                for _ in range(45):
                    nc.tensor.matmul(wps[:], wsrc[:], wsrc[:],
                                     start=True, stop=True)

            # Per-tensor streams on the 3 HWDGE queues (~26GB/s each),
            # chunked so early tiles unblock quickly:
            #   sync: kT, gpsimd: qT, scalar: wblk
            def chunked(eng, dst, src, width, edges):
                lo = 0
                for hi in edges:
                    hi = min(hi, width)
                    if hi > lo:
                        eng.dma_start(dst[:, lo:hi], src[:, lo:hi])
                    lo = hi
                if lo < width:
                    eng.dma_start(dst[:, lo:], src[:, lo:])
            chunked(nc.gpsimd, qT, qT_d, mh, [256, 1024, 4096])
            chunked(nc.scalar, wblk, wblk_d, wb_n, [256, 1024, 4096])
            if n > 1024:
                nc.sync.dma_start(kT[:, 1024:], kT_d[:, 1024:])

            def emit_mm1(g, hf, t):
                """mm1 for one mh-tile: c_per_half [128,512] psum tiles, each
                drained (relu+scale -> bf16) on a fixed engine per chunk."""
                tg = g * group_tiles + t
                qT_t = qT[:, bass.ts(tg, 128)]
                y_t = ypool.tile([128, n_chunk], bf16, tag="y")
                for c in range(c_per_half):
                    p1 = psum1.tile([128, 512], f32)
                    nc.tensor.matmul(
                        p1[:],
                        qT_t,
                        kT[:, bass.ds(hf * n_chunk + c * 512, 512)],
                        start=True,
                        stop=True,
                    )
                    ysl = y_t[:, bass.ts(c, 512)]
                    if t % 2 == 0:
                        nc.scalar.activation(
                            ysl, p1[:],
                            mybir.ActivationFunctionType.Relu,
                            scale=SOFTMAX_SCALE,
                        )
                    else:
                        nc.vector.tensor_scalar(
                            ysl, p1[:], SOFTMAX_SCALE, 0.0,
                            mybir.AluOpType.mult, mybir.AluOpType.max,
                        )
                return y_t

            def emit_mm2(p2_chunks, g, t, y_t):
                tg = g * group_tiles + t
                w_t = wblk[:, bass.ts(tg, gp)]
                for c in range(c_per_half):
                    nc.tensor.matmul(
                        p2_chunks[c][:],
                        w_t,
                        y_t[:, bass.ts(c, 512)],
                        start=(t == 0),
                        stop=(t == group_tiles - 1),
                    )

            DELAY = 3  # tiles of run-ahead before mm2 consumes a drained y

            def finish_pass(g, hf, p2_chunks):
                # per-chunk psum2 drain, alternating engines; stores on
                # two queues so the final store isn't one long DMA
                for c in range(c_per_half):
                    ost = ostage.tile([gp, 512], f32, tag="ost",
                                      name=f"ost_{g}_{hf}_{c}")
                    if (hf * c_per_half + c) % 2 == 0:
                        nc.vector.tensor_copy(ost[:], p2_chunks[c][:])
                    else:
                        nc.scalar.copy(ost[:], p2_chunks[c][:])
                    (nc.sync if c % 2 == 0 else nc.gpsimd).dma_start(
                        o_d[bass.ts(g, gp),
                            bass.ds(hf * n_chunk + c * 512, 512)],
                        ost[:],
                    )

            # Flat tile stream across all (group, half) passes with mm2
            # trailing DELAY tiles behind mm1 — the pipeline crosses pass
            # boundaries so the PE never drains at a boundary.
            passes = [(g, hf) for g in range(n_groups) for hf in range(n_halves)]
            stream = [(pi, t) for pi in range(len(passes))
                      for t in range(group_tiles)]
            p2_of = {}
            ys = {}
            for idx, (pi, t) in enumerate(stream):
                g, hf = passes[pi]
                ys[idx] = emit_mm1(g, hf, t)
                j = idx - DELAY
                if j >= 0:
                    pj, tj = stream[j]
                    gj, hfj = passes[pj]
                    if pj not in p2_of:
                        p2_of[pj] = [
                            psum2.tile([gp, 512], f32, tag="p2",
                                       name=f"p2_{gj}_{hfj}_{c}")
                            for c in range(c_per_half)
                        ]
                    emit_mm2(p2_of[pj], gj, tj, ys.pop(j))
                    if tj == group_tiles - 1:
                        finish_pass(gj, hfj, p2_of.pop(pj))
            for j in range(len(stream) - DELAY, len(stream)):
                pj, tj = stream[j]
                gj, hfj = passes[pj]
                if pj not in p2_of:
                    p2_of[pj] = [
                        psum2.tile([gp, 512], f32, tag="p2",
                                   name=f"p2_{gj}_{hfj}_{c}")
                        for c in range(c_per_half)
                    ]
                emit_mm2(p2_of[pj], gj, tj, ys.pop(j))
                if tj == group_tiles - 1:
                    finish_pass(gj, hfj, p2_of.pop(pj))

    nc.compile()
    return nc


def marshal_core_inputs(q, k, weights, core, m_loc=M_LOC, group_tiles=32):
    """Host-side layout marshalling for one core (no arithmetic)."""
    n_tiles = (m_loc * H) // 128
    gp = 4 * group_tiles
    bf16 = ml_dtypes.bfloat16

    q_sh = np.asarray(q[0, core * m_loc:(core + 1) * m_loc])   # (m_loc, H, D) bf16
    qT = np.ascontiguousarray(q_sh.reshape(m_loc * H, D).T)     # (128, mh)
    kT = np.ascontiguousarray(np.asarray(k[0]).T)               # (128, n)

    w_sh = np.asarray(weights[core * m_loc:(core + 1) * m_loc, 0, :])  # (m_loc, H)
    # wblk[row, tg*gp + col]: for tile tg (4 m's), local m j (0..3), head h:
    #   row = 32*j + h, col = 4*(tg % group_tiles) + j  -> w[m, h]
    wblk = np.zeros((n_tiles, 128, gp), dtype=bf16)
    w_r = w_sh.reshape(n_tiles, 4, H)                           # (tg, j, h)
    tgs = np.arange(n_tiles)
    for j in range(4):
        cols = 4 * (tgs % group_tiles) + j                      # (tg,)
        wblk[tgs[:, None], 32 * j + np.arange(H)[None, :], cols[:, None]] = w_r[:, j, :]
    wblk = np.ascontiguousarray(wblk.transpose(1, 0, 2).reshape(128, n_tiles * gp))

    return {"qT": qT, "kT": kT, "wblk": wblk}


_NC_CACHE = {}


def _get_nc():
    if "nc" not in _NC_CACHE:
        _NC_CACHE["nc"] = build_nc()
    return _NC_CACHE["nc"]


def kernel(q, k, weights):
    nc = _get_nc()
    in_maps = [marshal_core_inputs(q, k, weights, c) for c in range(N_CORES)]
    res = run_bass_kernel_spmd(nc, in_maps, list(range(N_CORES)))
    out = np.concatenate([res.results[c]["o"] for c in range(N_CORES)], axis=0)
    return out[None]  # (1, M, N) fp32


# revision 46
# speedup vs baseline: 1.0029x; 1.0029x over previous
"""Trainium2 Bass kernel for the BF16Indexer sparse-attention problem.

Computes, for B=1, M=2048, H=32, D=128, N=4096:
    logits = einsum('bmhd,bnd->bmhn', q, k)          (fp32 accum)
    o      = einsum('bmhn,bmh->bmn', relu(logits), w) / sqrt(D)

Sharding: M (query tokens) split across 8 cores; k replicated.

Per-core algorithm (M_loc = 256 rows, mh = M_loc*H = 8192):
  - qT  [128=d, mh]     (host-transposed shard of q)
  - kT  [128=d, N]      (host-transposed k, replicated)
  - wblk[128, n_tiles*128]  block-diagonal per-tile weight matrices
  - mm1 (PE):  for each mh-tile t (128 rows = 4 m's x 32 h):
        p1 = qT[:, t].T @ kT[:, chunk]         -> logits [128, 512] fp32 PSUM
  - drain (ACT on even tiles / DVE on odd): y = relu(scale*p1) -> bf16 SBUF
  - mm2 (PE):  p2[:, chunk] += wblk[:, t].T @ y  accumulated over the 32
        tiles of a group (block-diagonal lhsT routes each tile's 4 m's to
        the right 4 of 128 output partitions)
  - p2 [128=m, n_chunk] fp32 -> SBUF -> DMA to o[m, n]

The whole kernel is one flat software pipeline over (group, n-half, tile)
with mm2 trailing mm1 by DELAY tiles, so the PE streams matmuls
back-to-back (~215ns each) across pass boundaries. Steady state is
PE-bound at ~128 elem/cycle ingest for both matmuls (~220us/core); the
PSUM->SBUF relu drains run concurrently on ACT+DVE (~69% busy each).
PE warm-up matmuls trip the HAM clock gate to 2.4GHz during the initial
DMA loads.

kernel(**inputs) takes the FULL inputs and returns the FULL (1, 2048, 4096)
fp32 output; sharding/gather is host-side marshalling only (no host FLOPs).
Measured: ~242us HW exec per core (8 cores SPMD, PE 92% busy),
rel err 1.8e-3.
"""

import math
import numpy as np
import ml_dtypes

import concourse.bass as bass
import concourse.mybir as mybir
import concourse.tile as tile
from concourse import bacc
from concourse.bass_utils import run_bass_kernel_spmd

# Problem constants (hardcoded per harness contract)
B, M, H, D, N = 1, 2048, 32, 128, 4096
N_CORES = 8
M_LOC = M // N_CORES              # 256 query rows per core
MH = M_LOC * H                    # 8192
N_TILES = MH // 128               # 64 mh-tiles (4 m's each)
SOFTMAX_SCALE = 1.0 / math.sqrt(float(D))


def build_nc(m_loc=M_LOC, n=N, group_tiles=32, n_chunk=1024):
    """Build + compile the per-core bass program.

    group_tiles: mh-tiles per mm2 accumulation group (psum2 has
                 4*group_tiles output partitions).
    n_chunk:     n-columns processed per (group, half) pass; psum2 is
                 [128, n_chunk] fp32 = n_chunk/512 PSUM banks.
    """
    mh = m_loc * H
    n_tiles = mh // 128
    assert n_tiles % group_tiles == 0
    n_groups = n_tiles // group_tiles
    assert n % n_chunk == 0
    n_halves = n // n_chunk
    assert n_chunk % 512 == 0
    c_per_half = n_chunk // 512
    gp = 4 * group_tiles  # output partitions per group

    nc = bacc.Bacc("TRN2", target_bir_lowering=False, debug=False)

    bf16 = mybir.dt.bfloat16
    f32 = mybir.dt.float32

    qT_d = nc.dram_tensor("qT", [128, mh], bf16, kind="ExternalInput")
    kT_d = nc.dram_tensor("kT", [128, n], bf16, kind="ExternalInput")
    wblk_d = nc.dram_tensor("wblk", [128, n_tiles * gp], bf16, kind="ExternalInput")
    o_d = nc.dram_tensor("o", [m_loc, n], f32, kind="ExternalOutput")

    with tile.TileContext(nc) as tc:
        with (
            tc.tile_pool(name="const", bufs=1) as const_pool,
            tc.tile_pool(name="ypool", bufs=5) as ypool,
            tc.tile_pool(name="psum1", bufs=6, space="PSUM") as psum1,
            tc.tile_pool(name="psum2", bufs=2, space="PSUM") as psum2,
            tc.tile_pool(name="ostage", bufs=4) as ostage,
        ):
            qT = const_pool.tile([128, mh], bf16)
            kT = const_pool.tile([128, n], bf16)
            wblk = const_pool.tile([128, n_tiles * gp], bf16)

            wb_n = n_tiles * gp
            nc.sync.dma_start(kT[:, :512], kT_d[:, :512])
            nc.scalar.dma_start(kT[:, 512:1024], kT_d[:, 512:1024])
            # warm the ACT spline tables while DMAs run
            warm = const_pool.tile([128, 1], bf16)
            nc.gpsimd.memset(warm[:], 0)
            nc.scalar.activation(warm[:], warm[:],
                                 mybir.ActivationFunctionType.Relu)

            # warm the PE (HAM un-throttles after ~3.4us of activity) with
            # small matmuls on a zeroed scratch tile while DMAs run
            if n_tiles >= 16:
                wsrc = const_pool.tile([128, 128], bf16)
                nc.gpsimd.memset(wsrc[:], 0)
                wps = psum1.tile([128, 128], f32, tag="p1", name="warm_ps")
                for _ in range(45):
                    nc.tensor.matmul(wps[:], wsrc[:], wsrc[:],
                                     start=True, stop=True)

            # Per-tensor streams on the 3 HWDGE queues (~26GB/s each),
            # chunked so early tiles unblock quickly:
            #   sync: kT, gpsimd: qT, scalar: wblk
            def chunked(eng, dst, src, width, edges):
                lo = 0
                for hi in edges:
                    hi = min(hi, width)
                    if hi > lo:
                        eng.dma_start(dst[:, lo:hi], src[:, lo:hi])
                    lo = hi
                if lo < width:
                    eng.dma_start(dst[:, lo:], src[:, lo:])
            chunked(nc.gpsimd, qT, qT_d, mh, [256, 1024, 4096])
            chunked(nc.scalar, wblk, wblk_d, wb_n, [256, 1024, 4096])
            if n > 1024:
                nc.sync.dma_start(kT[:, 1024:], kT_d[:, 1024:])

            def emit_mm1(g, hf, t):
                """mm1 for one mh-tile: c_per_half [128,512] psum tiles, each
                drained (relu+scale -> bf16) on a fixed engine per chunk."""
                tg = g * group_tiles + t
                qT_t = qT[:, bass.ts(tg, 128)]
                y_t = ypool.tile([128, n_chunk], bf16, tag="y")
                for c in range(c_per_half):
                    p1 = psum1.tile([128, 512], f32)
                    nc.tensor.matmul(
                        p1[:],
                        qT_t,
                        kT[:, bass.ds(hf * n_chunk + c * 512, 512)],
                        start=True,
                        stop=True,
                    )
                    ysl = y_t[:, bass.ts(c, 512)]
                    if t % 2 == 0:
                        nc.scalar.activation(
                            ysl, p1[:],
                            mybir.ActivationFunctionType.Relu,
                            scale=SOFTMAX_SCALE,
                        )
                    else:
                        nc.vector.tensor_scalar(
                            ysl, p1[:], SOFTMAX_SCALE, 0.0,
                            mybir.AluOpType.mult, mybir.AluOpType.max,
                        )
                return y_t

            def emit_mm2(p2_chunks, g, t, y_t):
                tg = g * group_tiles + t
                w_t = wblk[:, bass.ts(tg, gp)]
                for c in range(c_per_half):
                    nc.tensor.matmul(
                        p2_chunks[c][:],
                        w_t,
                        y_t[:, bass.ts(c, 512)],
                        start=(t == 0),
                        stop=(t == group_tiles - 1),
                    )

            DELAY = 3  # tiles of run-ahead before mm2 consumes a drained y

            def finish_pass(g, hf, p2_chunks):
                # per-chunk psum2 drain, alternating engines; stores on
                # two queues so the final store isn't one long DMA
                for c in range(c_per_half):
                    ost = ostage.tile([gp, 512], f32, tag="ost",
                                      name=f"ost_{g}_{hf}_{c}")
                    if (hf * c_per_half + c) % 2 == 0:
                        nc.vector.tensor_copy(ost[:], p2_chunks[c][:])
                    else:
                        nc.scalar.copy(ost[:], p2_chunks[c][:])
                    (nc.sync if c % 2 == 0 else nc.scalar).dma_start(
                        o_d[bass.ts(g, gp),
                            bass.ds(hf * n_chunk + c * 512, 512)],
                        ost[:],
                    )

            # Flat tile stream across all (group, half) passes with mm2
            # trailing DELAY tiles behind mm1 — the pipeline crosses pass
            # boundaries so the PE never drains at a boundary.
            passes = [(g, hf) for g in range(n_groups) for hf in range(n_halves)]
            stream = [(pi, t) for pi in range(len(passes))
                      for t in range(group_tiles)]
            p2_of = {}
            ys = {}
            for idx, (pi, t) in enumerate(stream):
                g, hf = passes[pi]
                ys[idx] = emit_mm1(g, hf, t)
                j = idx - DELAY
                if j >= 0:
                    pj, tj = stream[j]
                    gj, hfj = passes[pj]
                    if pj not in p2_of:
                        p2_of[pj] = [
                            psum2.tile([gp, 512], f32, tag="p2",
                                       name=f"p2_{gj}_{hfj}_{c}")
                            for c in range(c_per_half)
                        ]
                    emit_mm2(p2_of[pj], gj, tj, ys.pop(j))
                    if tj == group_tiles - 1:
                        finish_pass(gj, hfj, p2_of.pop(pj))
            for j in range(len(stream) - DELAY, len(stream)):
                pj, tj = stream[j]
                gj, hfj = passes[pj]
                if pj not in p2_of:
                    p2_of[pj] = [
                        psum2.tile([gp, 512], f32, tag="p2",
                                   name=f"p2_{gj}_{hfj}_{c}")
                        for c in range(c_per_half)
                    ]
                emit_mm2(p2_of[pj], gj, tj, ys.pop(j))
                if tj == group_tiles - 1:
                    finish_pass(gj, hfj, p2_of.pop(pj))

    nc.compile()
    return nc


def marshal_core_inputs(q, k, weights, core, m_loc=M_LOC, group_tiles=32):
    """Host-side layout marshalling for one core (no arithmetic)."""
    n_tiles = (m_loc * H) // 128
    gp = 4 * group_tiles
    bf16 = ml_dtypes.bfloat16

    q_sh = np.asarray(q[0, core * m_loc:(core + 1) * m_loc])   # (m_loc, H, D) bf16
    qT = np.ascontiguousarray(q_sh.reshape(m_loc * H, D).T)     # (128, mh)
    kT = np.ascontiguousarray(np.asarray(k[0]).T)               # (128, n)

    w_sh = np.asarray(weights[core * m_loc:(core + 1) * m_loc, 0, :])  # (m_loc, H)
    # wblk[row, tg*gp + col]: for tile tg (4 m's), local m j (0..3), head h:
    #   row = 32*j + h, col = 4*(tg % group_tiles) + j  -> w[m, h]
    wblk = np.zeros((n_tiles, 128, gp), dtype=bf16)
    w_r = w_sh.reshape(n_tiles, 4, H)                           # (tg, j, h)
    tgs = np.arange(n_tiles)
    for j in range(4):
        cols = 4 * (tgs % group_tiles) + j                      # (tg,)
        wblk[tgs[:, None], 32 * j + np.arange(H)[None, :], cols[:, None]] = w_r[:, j, :]
    wblk = np.ascontiguousarray(wblk.transpose(1, 0, 2).reshape(128, n_tiles * gp))

    return {"qT": qT, "kT": kT, "wblk": wblk}


_NC_CACHE = {}


def _get_nc():
    if "nc" not in _NC_CACHE:
        _NC_CACHE["nc"] = build_nc()
    return _NC_CACHE["nc"]


def kernel(q, k, weights):
    nc = _get_nc()
    in_maps = [marshal_core_inputs(q, k, weights, c) for c in range(N_CORES)]
    res = run_bass_kernel_spmd(nc, in_maps, list(range(N_CORES)))
    out = np.concatenate([res.results[c]["o"] for c in range(N_CORES)], axis=0)
    return out[None]  # (1, M, N) fp32


# revision 47
# speedup vs baseline: 1.0110x; 1.0081x over previous
"""Trainium2 Bass kernel for the BF16Indexer sparse-attention problem.

Computes, for B=1, M=2048, H=32, D=128, N=4096:
    logits = einsum('bmhd,bnd->bmhn', q, k)          (fp32 accum)
    o      = einsum('bmhn,bmh->bmn', relu(logits), w) / sqrt(D)

Sharding: M (query tokens) split across 8 cores; k replicated.

Per-core algorithm (M_loc = 256 rows, mh = M_loc*H = 8192):
  - qT  [128=d, mh]     (host-transposed shard of q)
  - kT  [128=d, N]      (host-transposed k, replicated)
  - wblk[128, n_tiles*128]  block-diagonal per-tile weight matrices
  - mm1 (PE):  for each mh-tile t (128 rows = 4 m's x 32 h):
        p1 = qT[:, t].T @ kT[:, chunk]         -> logits [128, 512] fp32 PSUM
  - drain (ACT on even tiles / DVE on odd): y = relu(scale*p1) -> bf16 SBUF
  - mm2 (PE):  p2[:, chunk] += wblk[:, t].T @ y  accumulated over the 32
        tiles of a group (block-diagonal lhsT routes each tile's 4 m's to
        the right 4 of 128 output partitions)
  - p2 [128=m, n_chunk] fp32 -> SBUF -> DMA to o[m, n]

The whole kernel is one flat software pipeline over (group, n-half, tile)
with mm2 trailing mm1 by DELAY tiles, so the PE streams matmuls
back-to-back (~215ns each) across pass boundaries. Steady state is
PE-bound at ~128 elem/cycle ingest for both matmuls (~220us/core); the
PSUM->SBUF relu drains run concurrently on ACT+DVE (~69% busy each).
PE warm-up matmuls trip the HAM clock gate to 2.4GHz during the initial
DMA loads.

kernel(**inputs) takes the FULL inputs and returns the FULL (1, 2048, 4096)
fp32 output; sharding/gather is host-side marshalling only (no host FLOPs).
Measured: ~242us HW exec per core (8 cores SPMD, PE 92% busy),
rel err 1.8e-3.
"""

import math
import numpy as np
import ml_dtypes

import concourse.bass as bass
import concourse.mybir as mybir
import concourse.tile as tile
from concourse import bacc
from concourse.bass_utils import run_bass_kernel_spmd

# Problem constants (hardcoded per harness contract)
B, M, H, D, N = 1, 2048, 32, 128, 4096
N_CORES = 8
M_LOC = M // N_CORES              # 256 query rows per core
MH = M_LOC * H                    # 8192
N_TILES = MH // 128               # 64 mh-tiles (4 m's each)
SOFTMAX_SCALE = 1.0 / math.sqrt(float(D))


def build_nc(m_loc=M_LOC, n=N, group_tiles=32, n_chunk=1024):
    """Build + compile the per-core bass program.

    group_tiles: mh-tiles per mm2 accumulation group (psum2 has
                 4*group_tiles output partitions).
    n_chunk:     n-columns processed per (group, half) pass; psum2 is
                 [128, n_chunk] fp32 = n_chunk/512 PSUM banks.
    """
    mh = m_loc * H
    n_tiles = mh // 128
    assert n_tiles % group_tiles == 0
    n_groups = n_tiles // group_tiles
    assert n % n_chunk == 0
    n_halves = n // n_chunk
    assert n_chunk % 512 == 0
    c_per_half = n_chunk // 512
    gp = 4 * group_tiles  # output partitions per group

    nc = bacc.Bacc("TRN2", target_bir_lowering=False, debug=False)

    bf16 = mybir.dt.bfloat16
    f32 = mybir.dt.float32

    qT_d = nc.dram_tensor("qT", [128, mh], bf16, kind="ExternalInput")
    kT_d = nc.dram_tensor("kT", [128, n], bf16, kind="ExternalInput")
    wblk_d = nc.dram_tensor("wblk", [128, n_tiles * gp], bf16, kind="ExternalInput")
    o_d = nc.dram_tensor("o", [m_loc, n], f32, kind="ExternalOutput")

    with tile.TileContext(nc) as tc:
        with (
            tc.tile_pool(name="const", bufs=1) as const_pool,
            tc.tile_pool(name="ypool", bufs=5) as ypool,
            tc.tile_pool(name="psum1", bufs=6, space="PSUM") as psum1,
            tc.tile_pool(name="psum2", bufs=2, space="PSUM") as psum2,
            tc.tile_pool(name="ostage", bufs=4) as ostage,
        ):
            qT = const_pool.tile([128, mh], bf16)
            kT = const_pool.tile([128, n], bf16)
            wblk = const_pool.tile([128, n_tiles * gp], bf16)

            wb_n = n_tiles * gp
            nc.sync.dma_start(kT[:, :512], kT_d[:, :512])
            nc.scalar.dma_start(kT[:, 512:1024], kT_d[:, 512:1024])
            # warm the ACT spline tables while DMAs run
            warm = const_pool.tile([128, 1], bf16)
            nc.gpsimd.memset(warm[:], 0)
            nc.scalar.activation(warm[:], warm[:],
                                 mybir.ActivationFunctionType.Relu)

            # warm the PE (HAM un-throttles after ~3.4us of activity) with
            # small matmuls on a zeroed scratch tile while DMAs run
            if n_tiles >= 16:
                wsrc = const_pool.tile([128, 128], bf16)
                nc.gpsimd.memset(wsrc[:], 0)
                wps = psum1.tile([128, 128], f32, tag="p1", name="warm_ps")
                for _ in range(52):
                    nc.tensor.matmul(wps[:], wsrc[:], wsrc[:],
                                     start=True, stop=True)

            # Per-tensor streams on the 3 HWDGE queues (~26GB/s each),
            # chunked so early tiles unblock quickly:
            #   sync: kT, gpsimd: qT, scalar: wblk
            def chunked(eng, dst, src, width, edges):
                lo = 0
                for hi in edges:
                    hi = min(hi, width)
                    if hi > lo:
                        eng.dma_start(dst[:, lo:hi], src[:, lo:hi])
                    lo = hi
                if lo < width:
                    eng.dma_start(dst[:, lo:], src[:, lo:])
            chunked(nc.gpsimd, qT, qT_d, mh, [256, 1024, 4096])
            chunked(nc.scalar, wblk, wblk_d, wb_n, [256, 1024, 4096])
            if n > 1024:
                nc.sync.dma_start(kT[:, 1024:], kT_d[:, 1024:])

            def emit_mm1(g, hf, t):
                """mm1 for one mh-tile: c_per_half [128,512] psum tiles, each
                drained (relu+scale -> bf16) on a fixed engine per chunk."""
                tg = g * group_tiles + t
                qT_t = qT[:, bass.ts(tg, 128)]
                y_t = ypool.tile([128, n_chunk], bf16, tag="y")
                for c in range(c_per_half):
                    p1 = psum1.tile([128, 512], f32)
                    nc.tensor.matmul(
                        p1[:],
                        qT_t,
                        kT[:, bass.ds(hf * n_chunk + c * 512, 512)],
                        start=True,
                        stop=True,
                    )
                    ysl = y_t[:, bass.ts(c, 512)]
                    if t % 2 == 0:
                        nc.scalar.activation(
                            ysl, p1[:],
                            mybir.ActivationFunctionType.Relu,
                            scale=SOFTMAX_SCALE,
                        )
                    else:
                        nc.vector.tensor_scalar(
                            ysl, p1[:], SOFTMAX_SCALE, 0.0,
                            mybir.AluOpType.mult, mybir.AluOpType.max,
                        )
                return y_t

            def emit_mm2(p2_chunks, g, t, y_t):
                tg = g * group_tiles + t
                w_t = wblk[:, bass.ts(tg, gp)]
                for c in range(c_per_half):
                    nc.tensor.matmul(
                        p2_chunks[c][:],
                        w_t,
                        y_t[:, bass.ts(c, 512)],
                        start=(t == 0),
                        stop=(t == group_tiles - 1),
                    )

            DELAY = 3  # tiles of run-ahead before mm2 consumes a drained y

            def finish_pass(g, hf, p2_chunks):
                # per-chunk psum2 drain, alternating engines; stores on
                # two queues so the final store isn't one long DMA
                for c in range(c_per_half):
                    ost = ostage.tile([gp, 512], f32, tag="ost",
                                      name=f"ost_{g}_{hf}_{c}")
                    if (hf * c_per_half + c) % 2 == 0:
                        nc.vector.tensor_copy(ost[:], p2_chunks[c][:])
                    else:
                        nc.scalar.copy(ost[:], p2_chunks[c][:])
                    (nc.sync if c % 2 == 0 else nc.scalar).dma_start(
                        o_d[bass.ts(g, gp),
                            bass.ds(hf * n_chunk + c * 512, 512)],
                        ost[:],
                    )

            # Flat tile stream across all (group, half) passes with mm2
            # trailing DELAY tiles behind mm1 — the pipeline crosses pass
            # boundaries so the PE never drains at a boundary.
            passes = [(g, hf) for g in range(n_groups) for hf in range(n_halves)]
            stream = [(pi, t) for pi in range(len(passes))
                      for t in range(group_tiles)]
            p2_of = {}
            ys = {}
            for idx, (pi, t) in enumerate(stream):
                g, hf = passes[pi]
                ys[idx] = emit_mm1(g, hf, t)
                j = idx - DELAY
                if j >= 0:
                    pj, tj = stream[j]
                    gj, hfj = passes[pj]
                    if pj not in p2_of:
                        p2_of[pj] = [
                            psum2.tile([gp, 512], f32, tag="p2",
                                       name=f"p2_{gj}_{hfj}_{c}")
                            for c in range(c_per_half)
                        ]
                    emit_mm2(p2_of[pj], gj, tj, ys.pop(j))
                    if tj == group_tiles - 1:
                        finish_pass(gj, hfj, p2_of.pop(pj))
            for j in range(len(stream) - DELAY, len(stream)):
                pj, tj = stream[j]
                gj, hfj = passes[pj]
                if pj not in p2_of:
                    p2_of[pj] = [
                        psum2.tile([gp, 512], f32, tag="p2",
                                   name=f"p2_{gj}_{hfj}_{c}")
                        for c in range(c_per_half)
                    ]
                emit_mm2(p2_of[pj], gj, tj, ys.pop(j))
                if tj == group_tiles - 1:
                    finish_pass(gj, hfj, p2_of.pop(pj))

    nc.compile()
    return nc


def marshal_core_inputs(q, k, weights, core, m_loc=M_LOC, group_tiles=32):
    """Host-side layout marshalling for one core (no arithmetic)."""
    n_tiles = (m_loc * H) // 128
    gp = 4 * group_tiles
    bf16 = ml_dtypes.bfloat16

    q_sh = np.asarray(q[0, core * m_loc:(core + 1) * m_loc])   # (m_loc, H, D) bf16
    qT = np.ascontiguousarray(q_sh.reshape(m_loc * H, D).T)     # (128, mh)
    kT = np.ascontiguousarray(np.asarray(k[0]).T)               # (128, n)

    w_sh = np.asarray(weights[core * m_loc:(core + 1) * m_loc, 0, :])  # (m_loc, H)
    # wblk[row, tg*gp + col]: for tile tg (4 m's), local m j (0..3), head h:
    #   row = 32*j + h, col = 4*(tg % group_tiles) + j  -> w[m, h]
    wblk = np.zeros((n_tiles, 128, gp), dtype=bf16)
    w_r = w_sh.reshape(n_tiles, 4, H)                           # (tg, j, h)
    tgs = np.arange(n_tiles)
    for j in range(4):
        cols = 4 * (tgs % group_tiles) + j                      # (tg,)
        wblk[tgs[:, None], 32 * j + np.arange(H)[None, :], cols[:, None]] = w_r[:, j, :]
    wblk = np.ascontiguousarray(wblk.transpose(1, 0, 2).reshape(128, n_tiles * gp))

    return {"qT": qT, "kT": kT, "wblk": wblk}


_NC_CACHE = {}


def _get_nc():
    if "nc" not in _NC_CACHE:
        _NC_CACHE["nc"] = build_nc()
    return _NC_CACHE["nc"]


def kernel(q, k, weights):
    nc = _get_nc()
    in_maps = [marshal_core_inputs(q, k, weights, c) for c in range(N_CORES)]
    res = run_bass_kernel_spmd(nc, in_maps, list(range(N_CORES)))
    out = np.concatenate([res.results[c]["o"] for c in range(N_CORES)], axis=0)
    return out[None]  # (1, M, N) fp32
